# revision 13
# baseline (speedup 1.0000x reference)
"""BoundaryLoss Trainium2 kernel.

loss = mean(sigmoid(pred) * d),  d = sqrt(EDT2(mask==0)) - sqrt(EDT2(mask!=0))

per-mask exact Euclidean distance transform, computed separably:
  pass 1 (rows):   1-D distance to nearest source along each row, via two
                   min-plus scans (tensor_tensor_scan) -> g2 = g*g
  pass 2 (cols):   D2[i,j] = min_{|d|<=R} g2[i+d, j] + d*d, windowed with a
                   host-derived exact radius R (for every pixel the true
                   D2 <= R^2, so candidates beyond the window can never win)

Sharding: data-parallel over the B*C = 24 masks, 3 masks per core on 8 cores;
each core emits per-row partial sums of sigmoid(pred)*d, host reduces in f64.

Layout: the 3 masks x 2 polarities are packed side-by-side along the free
dimension with separator columns, so every vector op covers all 6 maps at
once.  Pass 2 runs on the transposed maps (PE transposes between passes) so
its shifts are free-dim slices.
"""

import numpy as np

import concourse.bass as bass
import concourse.tile as tile
from concourse import bacc, masks, mybir
from concourse.bass_utils import run_bass_kernel_spmd

H = W = 256
NMASK = 3            # masks per core
NCORES = 8
NB = 2 * NMASK       # packed blocks: 0..2 polarity "neg" (src mask!=0), 3..5 "pos" (src mask==0)
P1W = 7 + NB * 256   # pass-1 packed width (separator columns between/around blocks)
HUGE = float(2 ** 20)
INF2 = 65536.0       # pass-2 pad value (bf16-exact, beats any real candidate)

_NC_CACHE = {}


def _off1(b):
    return 1 + 257 * b


def build_nc(R):
    B2 = 256 + 2 * R
    dt = mybir.dt
    f32, bf16, i32 = dt.float32, dt.bfloat16, dt.int32
    AF = mybir.ActivationFunctionType
    OP = mybir.AluOpType

    nc = bacc.Bacc("TRN2", target_bir_lowering=False, debug=False, num_devices=NCORES)
    pred_h = nc.dram_tensor("pred", [NMASK, H, W], f32, kind="ExternalInput")
    targ_h = nc.dram_tensor("target", [2, 128, 773], i32, kind="ExternalInput")
    dscan_h = nc.dram_tensor("dscan", [128, P1W], f32, kind="ExternalInput")
    out_h = nc.dram_tensor("out", [128, 2], f32, kind="ExternalOutput")

    with tile.TileContext(nc) as tc:
        with (
            tc.tile_pool(name="const", bufs=1) as constp,
            tc.tile_pool(name="p1", bufs=2) as p1p,
            tc.tile_pool(name="p2", bufs=2) as p2p,
            tc.tile_pool(name="tail", bufs=2) as tailp,
            tc.tile_pool(name="psum", bufs=4, space="PSUM") as psp,
            tc.tile_pool(name="outp", bufs=1) as outp,
        ):
            ident = constp.tile([128, 128], bf16, tag="ident")
            masks.make_identity(nc, ident)
            dscan = constp.tile([128, P1W], f32, tag="dscan")
            nc.sync.dma_start(dscan, dscan_h.ap())

            targ_r = targ_h.ap()
            pred_r = pred_h.ap().rearrange("m (t p) j -> t p m j", p=128)

            # ---- pass 1: row distances, both polarities, packed [128, P1W]
            g2s = []
            for it in range(2):
                zi = p1p.tile([128, 773], i32, tag="zi")
                nc.sync.dma_start(zi[:], targ_r[it])
                zf = p1p.tile([128, P1W], f32, tag="zf")
                # upper half mirrors the lower-half block layout (offset 771)
                nc.vector.tensor_copy(zf[:, 0:772], zi[:, 0:772])
                nc.vector.tensor_copy(zf[:, 772:1543], zi[:, 1:772])

                # P1 = 0 at source pixels, ~-HUGE elsewhere (and at separators)
                p1 = p1p.tile([128, P1W], f32, tag="p1")
                nc.vector.tensor_scalar(
                    p1[:, 0:771], zf[:, 0:771], HUGE, -HUGE, op0=OP.mult, op1=OP.add
                )
                nc.vector.tensor_scalar_mul(p1[:, 771:1543], zf[:, 771:1543], -HUGE)
                sep_up = p1[:, 771:1542].rearrange("p (k w) -> p k w", w=257)[:, :, 0:1]
                nc.gpsimd.memset(sep_up, -HUGE)
                nc.gpsimd.memset(p1[:, 1542:1543], -HUGE)

                # s[t] = max(s[t-1] + dscan[t], P1[t]);  s = -(dist to source)
                sf = p1p.tile([128, P1W], f32, tag="sf")
                sr = p1p.tile([128, P1W], f32, tag="sr")
                nc.vector.tensor_tensor_scan(
                    sf[:], dscan[:], p1[:], -HUGE, op0=OP.add, op1=OP.max
                )
                nc.vector.tensor_tensor_scan(
                    sr[:, ::-1], dscan[:, ::-1], p1[:, ::-1], -HUGE,
                    op0=OP.add, op1=OP.max,
                )
                nc.vector.tensor_tensor(sf[:], sf[:], sr[:], op=OP.max)
                g2 = p1p.tile([128, P1W], bf16, tag="g2")
                nc.scalar.activation(g2[:], sf[:], AF.Square)
                g2s.append(g2)

            # ---- transpose g2 maps into padded pass-2 layout [128, 6*B2]
            t2s = []
            for jh in range(2):
                t2 = p2p.tile([128, NB * B2], bf16, tag="t2")
                nc.gpsimd.memset(t2[:], INF2)
                t2s.append(t2)
            for b in range(NB):
                for jh in range(2):
                    for it in range(2):
                        ps = psp.tile([128, 128], bf16, tag="ps")
                        src = g2s[it][:, _off1(b) + 128 * jh: _off1(b) + 128 * jh + 128]
                        nc.tensor.transpose(ps[:], src, ident[:])
                        dst = t2s[jh][:, b * B2 + R + 128 * it: b * B2 + R + 128 * it + 128]
                        nc.scalar.copy(dst, ps[:])

            # ---- pass 2: windowed min-plus over rows (now the free dim)
            es = []
            for jh in range(2):
                t2b = t2s[jh].rearrange("p (b w) -> p b w", w=B2)
                acc = p2p.tile([128, NB * 256], bf16, tag="acc")
                accb = acc.rearrange("p (b j) -> p b j", j=256)
                first = True
                for d in range(1, R + 1):
                    for s in (d, -d):
                        in1 = t2b[:, :, R:R + 256] if first else accb
                        first = False
                        nc.vector.scalar_tensor_tensor(
                            accb, t2b[:, :, R + s:R + s + 256], float(d * d), in1,
                            op0=OP.add, op1=OP.min,
                        )
                # e = pos2 - neg2 (exactly one of them is 0 at every pixel)
                e = p2p.tile([128, NMASK * 256], bf16, tag="e")
                nc.vector.tensor_tensor(
                    e[:], acc[:, 768:1536], acc[:, 0:768], op=OP.subtract
                )
                es.append(e)

            # ---- transpose e back to row-major layout
            ens = []
            for it in range(2):
                ens.append(tailp.tile([128, NMASK * 256], f32, tag="en", name=f"en{it}"))
            for mu in range(NMASK):
                for it in range(2):
                    for jh in range(2):
                        ps = psp.tile([128, 128], bf16, tag="ps")
                        src = es[jh][:, mu * 256 + 128 * it: mu * 256 + 128 * it + 128]
                        nc.tensor.transpose(ps[:], src, ident[:])
                        dst = ens[it][:, mu * 256 + 128 * jh: mu * 256 + 128 * jh + 128]
                        nc.scalar.copy(dst, ps[:])

            # ---- tail: d = sqrt(relu(e)) - sqrt(relu(-e)); accum sigmoid(pred)*d
            outsb = outp.tile([128, 2], f32, tag="outsb")
            for it in range(2):
                en = ens[it]
                pr = tailp.tile([128, NMASK * 256], f32, tag="pr")
                nc.sync.dma_start(pr[:], pred_r[it])
                sg = tailp.tile([128, NMASK * 256], f32, tag="sg")
                nc.scalar.activation(sg[:], pr[:], AF.Sigmoid)
                pa = tailp.tile([128, NMASK * 256], f32, tag="pa")
                nb_ = tailp.tile([128, NMASK * 256], f32, tag="nb")
                nc.scalar.activation(pa[:], en[:], AF.Relu)
                nc.scalar.activation(nb_[:], en[:], AF.Relu, scale=-1.0)
                nc.scalar.activation(pa[:], pa[:], AF.Sqrt)
                nc.scalar.activation(nb_[:], nb_[:], AF.Sqrt)
                dtl = tailp.tile([128, NMASK * 256], f32, tag="dtl")
                nc.vector.tensor_tensor(dtl[:], pa[:], nb_[:], op=OP.subtract)
                dm = tailp.tile([128, NMASK * 256], f32, tag="dm")
                nc.vector.tensor_tensor(dm[:], dtl[:], sg[:], op=OP.mult)
                scr = tailp.tile([128, NMASK * 256], f32, tag="scr")
                nc.scalar.activation(
                    scr[:], dm[:], AF.Copy, accum_out=outsb[:, it:it + 1]
                )
            nc.sync.dma_start(out_h.ap(), outsb[:])
    nc.compile()
    return nc


# ---------------------------------------------------------------------------
# host-side helpers

def _row_dist(src):
    """src: [N,H,W] bool. 1-D distance along W to nearest True (big if none)."""
    n, h, w = src.shape
    big = 10 ** 9
    col = np.arange(w)
    last = np.where(src, col, -big)
    np.maximum.accumulate(last, axis=2, out=last)
    nxt = np.where(src, col, big)
    nxt = np.minimum.accumulate(nxt[:, :, ::-1], axis=2)[:, :, ::-1]
    return np.minimum(np.minimum(col - last, nxt - col), big)


def _exact_d2(src):
    """src: [N,H,W] bool sources. Exact squared EDT (int64), via windowed
    min-plus with provably sufficient window."""
    g = _row_dist(src).astype(np.int64)
    g2 = np.minimum(g * g, 10 ** 14)
    d2 = g2.copy()
    cur_max = d2.max()
    for d in range(1, src.shape[1]):
        v = d * d
        if v > cur_max:
            break
        np.minimum(d2[:, d:, :], g2[:, :-d, :] + v, out=d2[:, d:, :])
        np.minimum(d2[:, :-d, :], g2[:, d:, :] + v, out=d2[:, :-d, :])
        cur_max = d2.max()
    return d2


def _host_loss_f64(pred24, z24):
    """Reference-faithful host fallback (only used for degenerate masks)."""
    d2n = _exact_d2(z24)
    d2p = _exact_d2(~z24)
    d = np.sqrt(d2p.astype(np.float64)) - np.sqrt(d2n.astype(np.float64))
    for m in range(z24.shape[0]):
        if not z24[m].any():
            d[m] = 0.0
    sig = 1.0 / (1.0 + np.exp(-pred24.astype(np.float64)))
    return np.float32((sig * d).mean())


def _pad_target(t3):
    """[3,256,256] int32 -> [2,128,773] int32, blocks at 1+257*b, zeros at
    separators (pass-1 packed layout, per row-half)."""
    out = np.zeros((2, 128, 773), dtype=np.int32)
    for b in range(NMASK):
        out[0, :, 1 + 257 * b:257 * b + 257] = t3[b, 0:128, :]
        out[1, :, 1 + 257 * b:257 * b + 257] = t3[b, 128:256, :]
    return out


def kernel(pred, target):
    pred24 = np.ascontiguousarray(np.asarray(pred, dtype=np.float32).reshape(24, H, W))
    targ24 = np.ascontiguousarray(np.asarray(target, dtype=np.int32).reshape(24, H, W))
    z24 = targ24 != 0

    degenerate = any((not z24[m].any()) or z24[m].all() for m in range(24))
    d2n = _exact_d2(z24)
    d2p = _exact_d2(~z24)
    if degenerate:
        return _host_loss_f64(pred24, z24)
    d2max = max(int(d2n.max()), int(d2p.max()))
    R = max(1, int(np.ceil(np.sqrt(d2max))))

    if R not in _NC_CACHE:
        _NC_CACHE[R] = build_nc(R)
    nc = _NC_CACHE[R]

    dscan = np.full((128, P1W), -1.0, dtype=np.float32)
    dscan[:, 0::257] = -HUGE

    in_maps = []
    for c in range(NCORES):
        in_maps.append({
            "pred": np.ascontiguousarray(pred24[3 * c:3 * c + 3]),
            "target": _pad_target(targ24[3 * c:3 * c + 3]),
            "dscan": dscan,
        })
    res = run_bass_kernel_spmd(nc, in_maps, core_ids=list(range(NCORES)))
    total = np.float64(0.0)
    for c in range(NCORES):
        total += np.asarray(res.results[c]["out"], dtype=np.float64).sum()
    return np.float32(total / (24.0 * H * W))


# revision 17
# speedup vs baseline: 1.2177x; 1.2177x over previous
"""BoundaryLoss Trainium2 kernel.

loss = mean(sigmoid(pred) * d),  d = sqrt(EDT2(mask==0)) - sqrt(EDT2(mask!=0))

Exact separable squared EDT per mask, both column pass and row pass expressed
as windowed min-plus chains (acc = min(acc, shifted + d^2)) with host-derived
exact per-slot window radii: for every pixel the true D2 <= W^2, so candidates
beyond the window can never win; candidates from pixels with no in-window
source carry INF and always lose.

Sharding: data-parallel over the B*C = 24 masks, 3 per core on 8 cores, masks
permuted so the largest-window masks share slot 0 (shift ops cover only the
chunk ranges that need them).  Each core returns per-row partial sums of
sigmoid(pred)*d; the host reduces in float64.

On-chip layout: 3 masks x 2 polarities x 2 row/col-halves = 12 independent
256-wide chunks packed along the free dimension; the two passes run on
PE-transposed data so every shift is a free-dim slice.
"""

import numpy as np

import concourse.tile as tile
from concourse import bacc, masks, mybir
from concourse.bass_utils import run_bass_kernel_spmd

H = W = 256
NMASK = 3
NCORES = 8
INF2 = 65536.0       # bf16-exact, absorbs +d^2, always loses to real candidates

_NC_CACHE = {}


def build_nc(wneg, wpos):
    """wneg/wpos: per-slot window radii (len 3, descending)."""
    wneg = list(wneg)
    wpos = list(wpos)
    maxw = max(wneg + wpos)
    CP = maxw               # pad width per chunk side
    B2 = 256 + 2 * CP
    dt = mybir.dt
    f32, bf16, i32 = dt.float32, dt.bfloat16, dt.int32
    AF = mybir.ActivationFunctionType
    OP = mybir.AluOpType

    nc = bacc.Bacc("TRN2", target_bir_lowering=False, debug=False, num_devices=NCORES)
    pred_h = nc.dram_tensor("pred", [NMASK, H, W], f32, kind="ExternalInput")
    targ_h = nc.dram_tensor("target", [NMASK, H, W], i32, kind="ExternalInput")
    out_h = nc.dram_tensor("out", [128, 1], f32, kind="ExternalOutput")

    # chunk k = pol*6 + slot*2 + half; windows per chunk follow (pol, slot)
    wchunk = wneg + wpos    # per (pol, slot)

    def shift_ranges(d):
        """chunk ranges [(k0, k1)] that still need shift distance d."""
        kn = sum(1 for s in range(NMASK) if wneg[s] >= d)
        kp = sum(1 for s in range(NMASK) if wpos[s] >= d)
        if kn == NMASK and kp == NMASK:
            return [(0, 12)]
        out = []
        if kn:
            out.append((0, 2 * kn))
        if kp:
            out.append((6, 6 + 2 * kp))
        return out

    def minplus_pass(nc, t2, acc, tag):
        """acc[chunk, j] = min_{|d|<=w_chunk} t2[chunk, CP+j+d] + d^2."""
        t2v = t2.rearrange("p (k w) -> p k w", w=B2)
        accv = acc.rearrange("p (k j) -> p k j", j=256)
        first = True
        for d in range(1, maxw + 1):
            for s in (d, -d):
                for (k0, k1) in shift_ranges(d):
                    in0 = t2v[:, k0:k1, CP + s:CP + s + 256]
                    acv = accv[:, k0:k1]
                    in1 = t2v[:, k0:k1, CP:CP + 256] if first else acv
                    nc.vector.scalar_tensor_tensor(
                        acv, in0, float(d * d), in1, op0=OP.add, op1=OP.min
                    )
                    first = False

    with tile.TileContext(nc) as tc:
        with (
            tc.tile_pool(name="const", bufs=1) as constp,
            tc.tile_pool(name="work", bufs=1) as wp,
            tc.tile_pool(name="psum", bufs=4, space="PSUM") as psp,
        ):
            ident = constp.tile([128, 128], bf16, tag="ident")
            masks.make_identity(nc, ident)

            targ_r = targ_h.ap().rearrange("m (t p) j -> t p m j", p=128)
            pred_r = pred_h.ap().rearrange("m (t p) j -> t p m j", p=128)

            # Z as bf16 0/1, per row-half, [slot, j] packed
            zbs = []
            for it in range(2):
                zi = wp.tile([128, NMASK * 256], i32, tag="zi", name=f"zi{it}", bufs=2)
                nc.sync.dma_start(zi[:], targ_r[it])
                zb = wp.tile([128, NMASK * 256], bf16, tag="zb", name=f"zb{it}", bufs=2)
                nc.vector.tensor_scalar_mul(zb[:], zi[:], 1.0)
                zbs.append(zb)

            # stage 1: transpose Z, write INF-maps for both polarities into t2
            t2 = wp.tile([128, 12 * B2], bf16, tag="t2")
            # pads <- INF (centers get overwritten)
            pv = t2.rearrange("p (k w) -> p k w", w=B2)
            nc.gpsimd.memset(pv[:, :, 0:CP], INF2)
            nc.gpsimd.memset(pv[:, :, CP + 256:B2], INF2)
            for s in range(NMASK):
                for jh in range(2):
                    ps = psp.tile([128, 256], bf16, tag="ps", name=f"ps{s}{jh}")
                    for it in range(2):
                        src = zbs[it][:, s * 256 + 128 * jh: s * 256 + 128 * jh + 128]
                        nc.tensor.transpose(ps[:, 128 * it:128 * it + 128], src, ident[:])
                    for pol in range(2):
                        k = pol * 6 + s * 2 + jh
                        dst = t2[:, k * B2 + CP: k * B2 + CP + 256]
                        if pol == 0:   # neg: sources Z==1 -> 0 where Z=1
                            nc.scalar.activation(dst, ps[:], AF.Copy,
                                                 scale=-INF2, bias=INF2)
                        else:          # pos: sources Z==0 -> 0 where Z=0
                            nc.scalar.activation(dst, ps[:], AF.Copy,
                                                 scale=INF2, bias=0.0)

            # pass A: column distances squared (shifts along i)
            acca = wp.tile([128, 12 * 256], bf16, tag="acca")
            minplus_pass(nc, t2, acca, "A")

            # stage 2: transpose gcol^2 back, pad along j
            t3 = wp.tile([128, 12 * B2], bf16, tag="t3")
            p3 = t3.rearrange("p (k w) -> p k w", w=B2)
            nc.gpsimd.memset(p3[:, :, 0:CP], INF2)
            nc.gpsimd.memset(p3[:, :, CP + 256:B2], INF2)
            for pol in range(2):
                for s in range(NMASK):
                    for ih in range(2):
                        ps2 = psp.tile([128, 256], bf16, tag="ps2",
                                       name=f"ps2_{pol}{s}{ih}")
                        for jh in range(2):
                            k1 = pol * 6 + s * 2 + jh
                            src = acca[:, k1 * 256 + 128 * ih: k1 * 256 + 128 * ih + 128]
                            nc.tensor.transpose(ps2[:, 128 * jh:128 * jh + 128],
                                                src, ident[:])
                        k2 = pol * 6 + s * 2 + ih
                        nc.scalar.copy(t3[:, k2 * B2 + CP: k2 * B2 + CP + 256], ps2[:])

            # pass B: full D2 (shifts along j)
            accb = wp.tile([128, 12 * 256], bf16, tag="accb")
            minplus_pass(nc, t3, accb, "B")

            # tail: d = sqrt(pos2) - sqrt(neg2); accumulate sigmoid(pred)*d
            sq = wp.tile([128, 12 * 256], f32, tag="sq")
            nc.scalar.activation(sq[:], accb[:], AF.Sqrt)
            dt_ = wp.tile([128, 6 * 256], f32, tag="dt_")
            nc.vector.tensor_tensor(dt_[:], sq[:, 6 * 256:12 * 256],
                                    sq[:, 0:6 * 256], op=OP.subtract)
            pr = wp.tile([128, 6 * 256], f32, tag="pr")
            prv = pr.rearrange("p (s t j) -> p s t j", s=NMASK, t=2)
            for it in range(2):
                nc.sync.dma_start(prv[:, :, it], pred_r[it])
            sg = wp.tile([128, 6 * 256], f32, tag="sg")
            nc.scalar.activation(sg[:], pr[:], AF.Sigmoid)
            dm = wp.tile([128, 6 * 256], f32, tag="dm")
            nc.vector.tensor_tensor(dm[:], dt_[:], sg[:], op=OP.mult)
            outsb = wp.tile([128, 1], f32, tag="outsb")
            scr = wp.tile([128, 6 * 256], f32, tag="scr")
            nc.scalar.activation(scr[:], dm[:], AF.Copy, accum_out=outsb[:])
            nc.sync.dma_start(out_h.ap(), outsb[:])
    nc.compile()
    return nc


# ---------------------------------------------------------------------------
# host side

def _row_dist(src):
    n, h, w = src.shape
    big = 10 ** 9
    col = np.arange(w)
    last = np.where(src, col, -big)
    np.maximum.accumulate(last, axis=2, out=last)
    nxt = np.where(src, col, big)
    nxt = np.minimum.accumulate(nxt[:, :, ::-1], axis=2)[:, :, ::-1]
    return np.minimum(np.minimum(col - last, nxt - col), big)


def _exact_d2(src):
    g = _row_dist(src).astype(np.int64)
    g2 = np.minimum(g * g, 10 ** 14)
    d2 = g2.copy()
    cur_max = d2.max()
    for d in range(1, src.shape[1]):
        v = d * d
        if v > cur_max:
            break
        np.minimum(d2[:, d:, :], g2[:, :-d, :] + v, out=d2[:, d:, :])
        np.minimum(d2[:, :-d, :], g2[:, d:, :] + v, out=d2[:, :-d, :])
        cur_max = d2.max()
    return d2


def _host_loss_f64(pred24, z24):
    d2n = _exact_d2(z24)
    d2p = _exact_d2(~z24)
    d = np.sqrt(d2p.astype(np.float64)) - np.sqrt(d2n.astype(np.float64))
    for m in range(z24.shape[0]):
        if not z24[m].any():
            d[m] = 0.0
    sig = 1.0 / (1.0 + np.exp(-pred24.astype(np.float64)))
    return np.float32((sig * d).mean())


def kernel(pred, target):
    pred24 = np.ascontiguousarray(np.asarray(pred, dtype=np.float32).reshape(24, H, W))
    targ24 = np.ascontiguousarray(np.asarray(target, dtype=np.int32).reshape(24, H, W))
    z24 = targ24 != 0

    if any((not z24[m].any()) or z24[m].all() for m in range(24)):
        return _host_loss_f64(pred24, z24)

    d2n = _exact_d2(z24).reshape(24, -1).max(1)
    d2p = _exact_d2(~z24).reshape(24, -1).max(1)
    wn = np.maximum(np.floor(np.sqrt(d2n)).astype(int), 1)
    wp = np.maximum(np.floor(np.sqrt(d2p)).astype(int), 1)

    # permute masks: largest windows first -> slot = rank // 8, core = rank % 8
    order = np.argsort(-np.maximum(wn, wp), kind="stable")
    slot_wn = [0] * NMASK
    slot_wp = [0] * NMASK
    for r, m in enumerate(order):
        s = r // NCORES
        slot_wn[s] = max(slot_wn[s], int(wn[m]))
        slot_wp[s] = max(slot_wp[s], int(wp[m]))
    # enforce descending (suffix windows may only shrink)
    for s in range(NMASK - 2, -1, -1):
        slot_wn[s] = max(slot_wn[s], slot_wn[s + 1])
        slot_wp[s] = max(slot_wp[s], slot_wp[s + 1])

    key = (tuple(slot_wn), tuple(slot_wp))
    if key not in _NC_CACHE:
        _NC_CACHE[key] = build_nc(slot_wn, slot_wp)
    nc = _NC_CACHE[key]

    in_maps = []
    for c in range(NCORES):
        midx = [order[s * NCORES + c] for s in range(NMASK)]
        in_maps.append({
            "pred": np.ascontiguousarray(pred24[midx]),
            "target": np.ascontiguousarray(targ24[midx]),
        })
    res = run_bass_kernel_spmd(nc, in_maps, core_ids=list(range(NCORES)))
    total = np.float64(0.0)
    for c in range(NCORES):
        total += np.asarray(res.results[c]["out"], dtype=np.float64).sum()
    return np.float32(total / (24.0 * H * W))


# revision 18
# speedup vs baseline: 1.3626x; 1.1191x over previous
"""BoundaryLoss Trainium2 kernel.

loss = mean(sigmoid(pred) * d),  d = sqrt(EDT2(mask==0)) - sqrt(EDT2(mask!=0))

Exact separable squared EDT per mask, both the column pass and the row pass
expressed as windowed min-plus chains (acc = min(acc, shifted + d^2)) with
host-derived exact window radii: for every pixel the true D2 <= W^2, so
candidates beyond the window can never win; pixels with no in-window source
carry INF and always lose.

Sharding: data-parallel over the B*C = 24 masks, 3 per core on 8 cores, masks
permuted so the largest-window masks land in slot 0.  Each slot (mask) forms
an independent pipeline: transpose Z -> pass A (column dist^2, shifts along
i) -> transpose -> pass B (full D2, shifts along j) -> sqrt/sigmoid/accumulate,
so the three slots overlap across engines.  Each core returns per-row partial
sums; the host reduces in float64.

Per-slot on-chip layout: 4 chunks (2 polarities x 2 halves) of one 256-wide
map packed along the free dimension, each padded with INF on both sides; all
shifts are free-dim slices on PE-transposed data.
"""

import numpy as np

import concourse.tile as tile
from concourse import bacc, masks, mybir
from concourse.bass_utils import run_bass_kernel_spmd

H = W = 256
NMASK = 3
NCORES = 8
INF2 = 65536.0       # bf16-exact, absorbs +d^2, always loses to real candidates

_NC_CACHE = {}


def build_nc(wslot):
    """wslot: per-slot window radii (len 3, descending)."""
    wslot = list(wslot)
    maxw = max(wslot)
    CP = maxw
    B2 = 256 + 2 * CP
    dt = mybir.dt
    f32, bf16, i32 = dt.float32, dt.bfloat16, dt.int32
    AF = mybir.ActivationFunctionType
    OP = mybir.AluOpType

    nc = bacc.Bacc("TRN2", target_bir_lowering=False, debug=False, num_devices=NCORES)
    pred_h = nc.dram_tensor("pred", [NMASK, H, W], f32, kind="ExternalInput")
    targ_h = nc.dram_tensor("target", [NMASK, H, W], i32, kind="ExternalInput")
    out_h = nc.dram_tensor("out", [128, NMASK], f32, kind="ExternalOutput")

    def minplus(nc, src, acc, w):
        """acc[k, j] = min_{|d|<=w} src[k, CP+j+d] + d^2 over 4 chunks."""
        sv = src.rearrange("p (k w) -> p k w", w=B2)
        av = acc.rearrange("p (k j) -> p k j", j=256)
        first = True
        for d in range(1, w + 1):
            for s in (d, -d):
                in1 = sv[:, :, CP:CP + 256] if first else av
                nc.vector.scalar_tensor_tensor(
                    av, sv[:, :, CP + s:CP + s + 256], float(d * d), in1,
                    op0=OP.add, op1=OP.min,
                )
                first = False

    with tile.TileContext(nc) as tc:
        with (
            tc.tile_pool(name="const", bufs=1) as constp,
            tc.tile_pool(name="work", bufs=1) as wp,
            tc.tile_pool(name="psum", bufs=4, space="PSUM") as psp,
        ):
            ident = constp.tile([128, 128], bf16, tag="ident")
            masks.make_identity(nc, ident)

            targ_r = targ_h.ap().rearrange("m (t p) j -> t p m j", p=128)

            # Z as bf16 0/1, per row-half, [slot, j] packed
            zbs = []
            for it in range(2):
                zi = wp.tile([128, NMASK * 256], i32, tag="zi", name=f"zi{it}", bufs=2)
                nc.sync.dma_start(zi[:], targ_r[it])
                zb = wp.tile([128, NMASK * 256], bf16, tag="zb", name=f"zb{it}", bufs=2)
                nc.vector.tensor_scalar_mul(zb[:], zi[:], 1.0)
                zbs.append(zb)

            outsb = wp.tile([128, NMASK], f32, tag="outsb")

            for s in range(NMASK):
                w = wslot[s]
                # ---- stage 1: transpose Z, write both polarity INF-maps
                t2 = wp.tile([128, 4 * B2], bf16, tag=f"t2_{s}", name=f"t2_{s}")
                pv = t2.rearrange("p (k w) -> p k w", w=B2)
                nc.gpsimd.memset(pv[:, :, 0:CP], INF2)
                nc.gpsimd.memset(pv[:, :, CP + 256:B2], INF2)
                for jh in range(2):
                    ps = psp.tile([128, 256], bf16, tag="ps", name=f"ps{s}{jh}")
                    for it in range(2):
                        src = zbs[it][:, s * 256 + 128 * jh: s * 256 + 128 * jh + 128]
                        nc.tensor.transpose(ps[:, 128 * it:128 * it + 128], src, ident[:])
                    for pol in range(2):
                        k = pol * 2 + jh
                        dst = t2[:, k * B2 + CP: k * B2 + CP + 256]
                        if pol == 0:   # neg: sources Z==1 -> 0 where Z=1
                            nc.scalar.activation(dst, ps[:], AF.Copy,
                                                 scale=-INF2, bias=INF2)
                        else:          # pos: sources Z==0 -> 0 where Z=0
                            nc.scalar.activation(dst, ps[:], AF.Copy,
                                                 scale=INF2, bias=0.0)

                # ---- pass A: column distances squared (shifts along i)
                acca = wp.tile([128, 4 * 256], bf16, tag=f"acca_{s}", name=f"acca_{s}")
                minplus(nc, t2, acca, w)

                # ---- stage 2: transpose gcol^2 back, pad along j
                t3 = wp.tile([128, 4 * B2], bf16, tag=f"t3_{s}", name=f"t3_{s}")
                p3 = t3.rearrange("p (k w) -> p k w", w=B2)
                nc.gpsimd.memset(p3[:, :, 0:CP], INF2)
                nc.gpsimd.memset(p3[:, :, CP + 256:B2], INF2)
                for pol in range(2):
                    for ih in range(2):
                        ps2 = psp.tile([128, 256], bf16, tag="ps2",
                                       name=f"ps2_{s}{pol}{ih}")
                        for jh in range(2):
                            k1 = pol * 2 + jh
                            src = acca[:, k1 * 256 + 128 * ih: k1 * 256 + 128 * ih + 128]
                            nc.tensor.transpose(ps2[:, 128 * jh:128 * jh + 128],
                                                src, ident[:])
                        k2 = pol * 2 + ih
                        nc.scalar.copy(t3[:, k2 * B2 + CP: k2 * B2 + CP + 256], ps2[:])

                # ---- pass B: full D2 (shifts along j)
                accb = wp.tile([128, 4 * 256], bf16, tag=f"accb_{s}", name=f"accb_{s}")
                minplus(nc, t3, accb, w)

                # ---- tail: d = sqrt(pos2) - sqrt(neg2); accum sigmoid(pred)*d
                sq = wp.tile([128, 4 * 256], f32, tag=f"sq_{s}", name=f"sq_{s}")
                nc.scalar.activation(sq[:], accb[:], AF.Sqrt)
                dt_ = wp.tile([128, 2 * 256], f32, tag=f"dt_{s}", name=f"dt_{s}")
                nc.vector.tensor_tensor(dt_[:], sq[:, 2 * 256:4 * 256],
                                        sq[:, 0:2 * 256], op=OP.subtract)
                pr = wp.tile([128, 2 * 256], f32, tag=f"pr_{s}", name=f"pr_{s}")
                # src pred[s, it*128+p, j] -> dst [p, (it, j)]
                pv2 = pred_h.ap()[s].rearrange("(t p) j -> p t j", p=128)
                nc.sync.dma_start(pr.rearrange("p (t j) -> p t j", t=2), pv2)
                sg = wp.tile([128, 2 * 256], f32, tag=f"sg_{s}", name=f"sg_{s}")
                nc.scalar.activation(sg[:], pr[:], AF.Sigmoid)
                dm = wp.tile([128, 2 * 256], f32, tag=f"dm_{s}", name=f"dm_{s}")
                nc.vector.tensor_tensor(dm[:], dt_[:], sg[:], op=OP.mult)
                scr = wp.tile([128, 2 * 256], f32, tag=f"scr_{s}", name=f"scr_{s}")
                nc.scalar.activation(scr[:], dm[:], AF.Copy,
                                     accum_out=outsb[:, s:s + 1])

            nc.sync.dma_start(out_h.ap(), outsb[:])
    nc.compile()
    return nc


# ---------------------------------------------------------------------------
# host side

def _row_dist(src):
    n, h, w = src.shape
    big = 10 ** 9
    col = np.arange(w)
    last = np.where(src, col, -big)
    np.maximum.accumulate(last, axis=2, out=last)
    nxt = np.where(src, col, big)
    nxt = np.minimum.accumulate(nxt[:, :, ::-1], axis=2)[:, :, ::-1]
    return np.minimum(np.minimum(col - last, nxt - col), big)


def _exact_d2(src):
    g = _row_dist(src).astype(np.int64)
    g2 = np.minimum(g * g, 10 ** 14)
    d2 = g2.copy()
    cur_max = d2.max()
    for d in range(1, src.shape[1]):
        v = d * d
        if v > cur_max:
            break
        np.minimum(d2[:, d:, :], g2[:, :-d, :] + v, out=d2[:, d:, :])
        np.minimum(d2[:, :-d, :], g2[:, d:, :] + v, out=d2[:, :-d, :])
        cur_max = d2.max()
    return d2


def _host_loss_f64(pred24, z24):
    d2n = _exact_d2(z24)
    d2p = _exact_d2(~z24)
    d = np.sqrt(d2p.astype(np.float64)) - np.sqrt(d2n.astype(np.float64))
    for m in range(z24.shape[0]):
        if not z24[m].any():
            d[m] = 0.0
    sig = 1.0 / (1.0 + np.exp(-pred24.astype(np.float64)))
    return np.float32((sig * d).mean())


def _plan(targ24):
    """Returns (per-slot windows [3], mask order [24])."""
    z24 = targ24 != 0
    d2n = _exact_d2(z24).reshape(24, -1).max(1)
    d2p = _exact_d2(~z24).reshape(24, -1).max(1)
    wn = np.maximum(np.floor(np.sqrt(d2n)).astype(int), 1)
    wp_ = np.maximum(np.floor(np.sqrt(d2p)).astype(int), 1)
    wm = np.maximum(wn, wp_)
    order = np.argsort(-wm, kind="stable")
    wslot = [0] * NMASK
    for r, m in enumerate(order):
        s = r // NCORES
        wslot[s] = max(wslot[s], int(wm[m]))
    for s in range(NMASK - 2, -1, -1):
        wslot[s] = max(wslot[s], wslot[s + 1])
    return wslot, order


def kernel(pred, target):
    pred24 = np.ascontiguousarray(np.asarray(pred, dtype=np.float32).reshape(24, H, W))
    targ24 = np.ascontiguousarray(np.asarray(target, dtype=np.int32).reshape(24, H, W))
    z24 = targ24 != 0

    if any((not z24[m].any()) or z24[m].all() for m in range(24)):
        return _host_loss_f64(pred24, z24)

    wslot, order = _plan(targ24)
    key = tuple(wslot)
    if key not in _NC_CACHE:
        _NC_CACHE[key] = build_nc(wslot)
    nc = _NC_CACHE[key]

    in_maps = []
    for c in range(NCORES):
        midx = [order[s * NCORES + c] for s in range(NMASK)]
        in_maps.append({
            "pred": np.ascontiguousarray(pred24[midx]),
            "target": np.ascontiguousarray(targ24[midx]),
        })
    res = run_bass_kernel_spmd(nc, in_maps, core_ids=list(range(NCORES)))
    total = np.float64(0.0)
    for c in range(NCORES):
        total += np.asarray(res.results[c]["out"], dtype=np.float64).sum()
    return np.float32(total / (24.0 * H * W))


# revision 22
# speedup vs baseline: 1.4554x; 1.0681x over previous
"""BoundaryLoss Trainium2 kernel.

loss = mean(sigmoid(pred) * d),  d = sqrt(EDT2(mask==0)) - sqrt(EDT2(mask!=0))

Exact separable squared EDT per mask, both the column pass and the row pass
expressed as windowed min-plus chains (acc = min(acc, shifted + d^2)) with
host-derived exact window radii: for every pixel the true D2 <= W^2, so
candidates beyond the window can never win; pixels with no in-window source
carry INF and always lose.

Sharding: data-parallel over the B*C = 24 masks, 3 per core on 8 cores, masks
permuted so the largest-window masks land in slot 0.  Each slot (mask) forms
an independent pipeline: transpose Z -> pass A (column dist^2, shifts along
i) -> transpose -> pass B (full D2, shifts along j) -> sqrt/sigmoid/accumulate,
so the three slots overlap across engines.  Each core returns per-row partial
sums; the host reduces in float64.

Per-slot on-chip layout: 4 chunks (2 polarities x 2 halves) of one 256-wide
map packed along the free dimension, each padded with INF on both sides; all
shifts are free-dim slices on PE-transposed data.
"""

import numpy as np

import concourse.tile as tile
from concourse import bacc, masks, mybir
from concourse.bass_utils import run_bass_kernel_spmd

H = W = 256
NMASK = 3
NCORES = 8
INF2 = 65536.0       # bf16-exact, absorbs +d^2, always loses to real candidates

_NC_CACHE = {}


def build_nc(wneg, wpos):
    """wneg/wpos: per-slot per-polarity window radii (len 3, descending)."""
    wneg = list(wneg)
    wpos = list(wpos)
    maxw = max(wneg + wpos)
    CP = maxw
    B2 = 256 + 2 * CP
    dt = mybir.dt
    f32, bf16, i32 = dt.float32, dt.bfloat16, dt.int32
    AF = mybir.ActivationFunctionType
    OP = mybir.AluOpType

    nc = bacc.Bacc("TRN2", target_bir_lowering=False, debug=False, num_devices=NCORES)
    pred_h = nc.dram_tensor("pred", [NMASK, H, W], f32, kind="ExternalInput")
    targ_h = nc.dram_tensor("target", [NMASK, H, W], i32, kind="ExternalInput")
    out_h = nc.dram_tensor("out", [128, NMASK], f32, kind="ExternalOutput")

    def minplus(nc, src, acc, wn, wp):
        """acc[k, j] = min_{|d|<=w_k} src[k, CP+j+d] + d^2; chunks 0-1 neg
        (window wn), chunks 2-3 pos (window wp >= wn)."""
        sv = src.rearrange("p (k w) -> p k w", w=B2)
        av = acc.rearrange("p (k j) -> p k j", j=256)
        first = True
        for d in range(1, wp + 1):
            k0 = 0 if d <= wn else 2
            for s in (d, -d):
                in1 = sv[:, k0:4, CP:CP + 256] if first else av[:, k0:4]
                nc.vector.scalar_tensor_tensor(
                    av[:, k0:4], sv[:, k0:4, CP + s:CP + s + 256], float(d * d),
                    in1, op0=OP.add, op1=OP.min,
                )
                first = False

    with tile.TileContext(nc) as tc:
        with (
            tc.tile_pool(name="const", bufs=1) as constp,
            tc.tile_pool(name="work", bufs=1) as wp,
            tc.tile_pool(name="psum", bufs=4, space="PSUM") as psp,
        ):
            ident = constp.tile([128, 128], bf16, tag="ident")
            masks.make_identity(nc, ident)

            targ_r = targ_h.ap().rearrange("m (t p) j -> m t p j", p=128)

            # Z as bf16 0/1, per row-half, [slot, j] packed; per-slot DMAs so
            # each slot's pipeline starts as soon as its data lands
            zbs = []
            for it in range(2):
                zb = wp.tile([128, NMASK * 256], bf16, tag="zb", name=f"zb{it}", bufs=2)
                for s in range(NMASK):
                    zi = wp.tile([128, 256], i32, tag="zi",
                                 name=f"zi{it}{s}", bufs=6)
                    nc.sync.dma_start(zi[:], targ_r[s, it])
                    nc.vector.tensor_scalar_mul(
                        zb[:, s * 256:s * 256 + 256], zi[:], 1.0)
                zbs.append(zb)

            outsb = wp.tile([128, NMASK], f32, tag="outsb")

            for s in range(NMASK):
                wn_, wp_ = wneg[s], wpos[s]
                # ---- stage 1: transpose Z, write both polarity INF-maps
                t2 = wp.tile([128, 4 * B2], bf16, tag=f"t2_{s}", name=f"t2_{s}")
                pv = t2.rearrange("p (k w) -> p k w", w=B2)
                nc.gpsimd.memset(pv[:, :, 0:CP], INF2)
                nc.gpsimd.memset(pv[:, :, CP + 256:B2], INF2)
                for jh in range(2):
                    ps = psp.tile([128, 256], bf16, tag="ps", name=f"ps{s}{jh}")
                    for it in range(2):
                        src = zbs[it][:, s * 256 + 128 * jh: s * 256 + 128 * jh + 128]
                        nc.tensor.transpose(ps[:, 128 * it:128 * it + 128], src, ident[:])
                    for pol in range(2):
                        k = pol * 2 + jh
                        dst = t2[:, k * B2 + CP: k * B2 + CP + 256]
                        if pol == 0:   # neg: sources Z==1 -> 0 where Z=1
                            nc.scalar.activation(dst, ps[:], AF.Copy,
                                                 scale=-INF2, bias=INF2)
                        else:          # pos: sources Z==0 -> 0 where Z=0
                            nc.scalar.activation(dst, ps[:], AF.Copy,
                                                 scale=INF2, bias=0.0)

                # ---- pass A: column distances squared (shifts along i)
                acca = wp.tile([128, 4 * 256], bf16, tag=f"acca_{s}", name=f"acca_{s}")
                minplus(nc, t2, acca, wn_, wp_)

                # ---- stage 2: transpose gcol^2 back, pad along j
                t3 = wp.tile([128, 4 * B2], bf16, tag=f"t3_{s}", name=f"t3_{s}")
                p3 = t3.rearrange("p (k w) -> p k w", w=B2)
                nc.gpsimd.memset(p3[:, :, 0:CP], INF2)
                nc.gpsimd.memset(p3[:, :, CP + 256:B2], INF2)
                for pol in range(2):
                    for ih in range(2):
                        ps2 = psp.tile([128, 256], bf16, tag="ps2",
                                       name=f"ps2_{s}{pol}{ih}")
                        for jh in range(2):
                            k1 = pol * 2 + jh
                            src = acca[:, k1 * 256 + 128 * ih: k1 * 256 + 128 * ih + 128]
                            nc.tensor.transpose(ps2[:, 128 * jh:128 * jh + 128],
                                                src, ident[:])
                        k2 = pol * 2 + ih
                        nc.scalar.copy(t3[:, k2 * B2 + CP: k2 * B2 + CP + 256], ps2[:])

                # ---- pass B: full D2 (shifts along j)
                accb = wp.tile([128, 4 * 256], bf16, tag=f"accb_{s}", name=f"accb_{s}")
                minplus(nc, t3, accb, wn_, wp_)

                # ---- tail: d = sqrt(pos2) - sqrt(neg2); accum sigmoid(pred)*d
                sq = wp.tile([128, 4 * 256], f32, tag=f"sq_{s}", name=f"sq_{s}")
                nc.scalar.activation(sq[:], accb[:], AF.Sqrt)
                dt_ = wp.tile([128, 2 * 256], f32, tag=f"dt_{s}", name=f"dt_{s}")
                nc.vector.tensor_tensor(dt_[:], sq[:, 2 * 256:4 * 256],
                                        sq[:, 0:2 * 256], op=OP.subtract)
                pr = wp.tile([128, 2 * 256], f32, tag=f"pr_{s}", name=f"pr_{s}")
                # src pred[s, it*128+p, j] -> dst [p, (it, j)]
                pv2 = pred_h.ap()[s].rearrange("(t p) j -> p t j", p=128)
                nc.sync.dma_start(pr.rearrange("p (t j) -> p t j", t=2), pv2)
                sg = wp.tile([128, 2 * 256], f32, tag=f"sg_{s}", name=f"sg_{s}")
                nc.scalar.activation(sg[:], pr[:], AF.Sigmoid)
                dm = wp.tile([128, 2 * 256], f32, tag=f"dm_{s}", name=f"dm_{s}")
                nc.vector.tensor_tensor(dm[:], dt_[:], sg[:], op=OP.mult)
                scr = wp.tile([128, 2 * 256], f32, tag=f"scr_{s}", name=f"scr_{s}")
                nc.scalar.activation(scr[:], dm[:], AF.Copy,
                                     accum_out=outsb[:, s:s + 1])

            nc.sync.dma_start(out_h.ap(), outsb[:])
    nc.compile()
    return nc


# ---------------------------------------------------------------------------
# host side

def _row_dist(src):
    n, h, w = src.shape
    big = 10 ** 9
    col = np.arange(w)
    last = np.where(src, col, -big)
    np.maximum.accumulate(last, axis=2, out=last)
    nxt = np.where(src, col, big)
    nxt = np.minimum.accumulate(nxt[:, :, ::-1], axis=2)[:, :, ::-1]
    return np.minimum(np.minimum(col - last, nxt - col), big)


def _exact_d2(src):
    g = _row_dist(src).astype(np.int64)
    g2 = np.minimum(g * g, 10 ** 14)
    d2 = g2.copy()
    cur_max = d2.max()
    for d in range(1, src.shape[1]):
        v = d * d
        if v > cur_max:
            break
        np.minimum(d2[:, d:, :], g2[:, :-d, :] + v, out=d2[:, d:, :])
        np.minimum(d2[:, :-d, :], g2[:, d:, :] + v, out=d2[:, :-d, :])
        cur_max = d2.max()
    return d2


def _host_loss_f64(pred24, z24):
    d2n = _exact_d2(z24)
    d2p = _exact_d2(~z24)
    d = np.sqrt(d2p.astype(np.float64)) - np.sqrt(d2n.astype(np.float64))
    for m in range(z24.shape[0]):
        if not z24[m].any():
            d[m] = 0.0
    sig = 1.0 / (1.0 + np.exp(-pred24.astype(np.float64)))
    return np.float32((sig * d).mean())


def _plan(targ24):
    """Returns (per-slot neg windows, pos windows, mask order)."""
    z24 = targ24 != 0
    d2n = _exact_d2(z24).reshape(24, -1).max(1)
    d2p = _exact_d2(~z24).reshape(24, -1).max(1)
    wn = np.maximum(np.floor(np.sqrt(d2n)).astype(int), 1)
    wp_ = np.maximum(np.floor(np.sqrt(d2p)).astype(int), 1)
    wm = np.maximum(wn, wp_)
    order = np.argsort(-wm, kind="stable")
    swn = [0] * NMASK
    swp = [0] * NMASK
    for r, m in enumerate(order):
        s = r // NCORES
        swn[s] = max(swn[s], int(wn[m]))
        swp[s] = max(swp[s], int(wp_[m]))
    for s in range(NMASK - 2, -1, -1):
        swn[s] = max(swn[s], swn[s + 1])
        swp[s] = max(swp[s], swp[s + 1])
    # kernel assumes wpos >= wneg per slot (pos-only tail shifts)
    for s in range(NMASK):
        if swn[s] > swp[s]:
            swn[s], swp[s] = swp[s], swn[s]
    return swn, swp, order


def kernel(pred, target):
    pred24 = np.ascontiguousarray(np.asarray(pred, dtype=np.float32).reshape(24, H, W))
    targ24 = np.ascontiguousarray(np.asarray(target, dtype=np.int32).reshape(24, H, W))
    z24 = targ24 != 0

    if any((not z24[m].any()) or z24[m].all() for m in range(24)):
        return _host_loss_f64(pred24, z24)

    swn, swp, order = _plan(targ24)
    key = (tuple(swn), tuple(swp))
    if key not in _NC_CACHE:
        _NC_CACHE[key] = build_nc(swn, swp)
    nc = _NC_CACHE[key]

    in_maps = []
    for c in range(NCORES):
        midx = [order[s * NCORES + c] for s in range(NMASK)]
        in_maps.append({
            "pred": np.ascontiguousarray(pred24[midx]),
            "target": np.ascontiguousarray(targ24[midx]),
        })
    res = run_bass_kernel_spmd(nc, in_maps, core_ids=list(range(NCORES)))
    total = np.float64(0.0)
    for c in range(NCORES):
        total += np.asarray(res.results[c]["out"], dtype=np.float64).sum()
    return np.float32(total / (24.0 * H * W))


# revision 24
# speedup vs baseline: 1.5516x; 1.0661x over previous
"""BoundaryLoss Trainium2 kernel.

loss = mean(sigmoid(pred) * d),  d = sqrt(EDT2(mask==0)) - sqrt(EDT2(mask!=0))

Exact separable squared EDT per mask, both the column pass and the row pass
expressed as windowed min-plus chains (acc = min(acc, shifted + d^2)) with
host-derived exact window radii: for every pixel the true D2 <= W^2, so
candidates beyond the window can never win; pixels with no in-window source
carry INF and always lose.

Sharding: data-parallel over the B*C = 24 masks, 3 per core on 8 cores, masks
permuted so the largest-window masks land in slot 0.  Each slot (mask) forms
an independent pipeline: transpose Z -> pass A (column dist^2, shifts along
i) -> transpose -> pass B (full D2, shifts along j) -> sqrt/sigmoid/accumulate,
so the three slots overlap across engines.  Each core returns per-row partial
sums; the host reduces in float64.

Per-slot on-chip layout: 4 chunks (2 polarities x 2 halves) of one 256-wide
map packed along the free dimension, each padded with INF on both sides; all
shifts are free-dim slices on PE-transposed data.
"""

import numpy as np

import concourse.tile as tile
from concourse import bacc, masks, mybir
from concourse.tile_rust import add_dep_helper
from concourse.bass_utils import run_bass_kernel_spmd

H = W = 256
NMASK = 3
NCORES = 8
INF2 = 65536.0       # bf16-exact, absorbs +d^2, always loses to real candidates

_NC_CACHE = {}


def build_nc(wneg, wpos):
    """wneg/wpos: per-slot per-polarity window radii (len 3, descending)."""
    wneg = list(wneg)
    wpos = list(wpos)
    maxw = max(wneg + wpos)
    CP = maxw
    B2 = 256 + 2 * CP
    dt = mybir.dt
    f32, bf16, i32 = dt.float32, dt.bfloat16, dt.int32
    AF = mybir.ActivationFunctionType
    OP = mybir.AluOpType

    nc = bacc.Bacc("TRN2", target_bir_lowering=False, debug=False, num_devices=NCORES)
    pred_h = nc.dram_tensor("pred", [NMASK, H, W], f32, kind="ExternalInput")
    targ_h = nc.dram_tensor("target", [NMASK, H, W], i32, kind="ExternalInput")
    out_h = nc.dram_tensor("out", [128, NMASK], f32, kind="ExternalOutput")

    def minplus(nc, pool, src, acc, wn, wp, tag):
        """acc[k, j] = min_{|d|<=w_k} src[k, CP+j+d] + d^2; chunks 0-1 neg
        (window wn), chunks 2-3 pos (window wp >= wn).  For shifts where both
        slice starts are even (bf16 2x eligible), pair +d/-d through a dense
        tensor_tensor min first.  Returns the last instruction."""
        sv = src.rearrange("p (k w) -> p k w", w=B2)
        av = acc.rearrange("p (k j) -> p k j", j=256)
        first = True
        last = None
        for d in range(1, wp + 1):
            k0 = 0 if d <= wn else 2
            nk = 4 - k0
            in1c = sv[:, k0:4, CP:CP + 256]
            if (CP + d) % 2 == 0:
                md = pool.tile([128, nk * 256], mybir.dt.bfloat16,
                               tag=f"md{tag}", name=f"md{tag}_{d}", bufs=2)
                mdv = md.rearrange("p (k j) -> p k j", j=256)
                nc.vector.tensor_tensor(
                    mdv, sv[:, k0:4, CP + d:CP + d + 256],
                    sv[:, k0:4, CP - d:CP - d + 256], op=OP.min,
                )
                in1 = in1c if first else av[:, k0:4]
                last = nc.vector.scalar_tensor_tensor(
                    av[:, k0:4], mdv, float(d * d), in1, op0=OP.add, op1=OP.min
                )
                first = False
            else:
                for s in (d, -d):
                    in1 = in1c if first else av[:, k0:4]
                    last = nc.vector.scalar_tensor_tensor(
                        av[:, k0:4], sv[:, k0:4, CP + s:CP + s + 256],
                        float(d * d), in1, op0=OP.add, op1=OP.min,
                    )
                    first = False
        return last

    with tile.TileContext(nc) as tc:
        with (
            tc.tile_pool(name="const", bufs=1) as constp,
            tc.tile_pool(name="work", bufs=1) as wp,
            tc.tile_pool(name="psum", bufs=4, space="PSUM") as psp,
        ):
            ident = constp.tile([128, 128], bf16, tag="ident")
            masks.make_identity(nc, ident)

            targ_r = targ_h.ap().rearrange("m (t p) j -> m t p j", p=128)

            # Z as bf16 0/1, per row-half, [slot, j] packed; per-slot DMAs so
            # each slot's pipeline starts as soon as its data lands
            zbs = []
            for it in range(2):
                zb = wp.tile([128, NMASK * 256], bf16, tag="zb", name=f"zb{it}", bufs=2)
                for s in range(NMASK):
                    zi = wp.tile([128, 256], i32, tag="zi",
                                 name=f"zi{it}{s}", bufs=6)
                    nc.sync.dma_start(zi[:], targ_r[s, it])
                    nc.vector.tensor_scalar_mul(
                        zb[:, s * 256:s * 256 + 256], zi[:], 1.0)
                zbs.append(zb)

            outsb = wp.tile([128, NMASK], f32, tag="outsb")

            for s in range(NMASK):
                wn_, wp_ = wneg[s], wpos[s]
                # ---- stage 1: transpose Z, write both polarity INF-maps
                t2 = wp.tile([128, 4 * B2], bf16, tag=f"t2_{s}", name=f"t2_{s}")
                pv = t2.rearrange("p (k w) -> p k w", w=B2)
                nc.gpsimd.memset(pv[:, :, 0:CP], INF2)
                nc.gpsimd.memset(pv[:, :, CP + 256:B2], INF2)
                for jh in range(2):
                    ps = psp.tile([128, 256], bf16, tag="ps", name=f"ps{s}{jh}")
                    for it in range(2):
                        src = zbs[it][:, s * 256 + 128 * jh: s * 256 + 128 * jh + 128]
                        nc.tensor.transpose(ps[:, 128 * it:128 * it + 128], src, ident[:])
                    for pol in range(2):
                        k = pol * 2 + jh
                        dst = t2[:, k * B2 + CP: k * B2 + CP + 256]
                        if pol == 0:   # neg: sources Z==1 -> 0 where Z=1
                            nc.scalar.activation(dst, ps[:], AF.Copy,
                                                 scale=-INF2, bias=INF2)
                        else:          # pos: sources Z==0 -> 0 where Z=0
                            nc.scalar.activation(dst, ps[:], AF.Copy,
                                                 scale=INF2, bias=0.0)

                # ---- pass A: column distances squared (shifts along i)
                acca = wp.tile([128, 4 * 256], bf16, tag=f"acca_{s}", name=f"acca_{s}")
                last_a = minplus(nc, wp, t2, acca, wn_, wp_, f"a{s}")

                # ---- stage 2: transpose gcol^2 back, pad along j
                t3 = wp.tile([128, 4 * B2], bf16, tag=f"t3_{s}", name=f"t3_{s}")
                p3 = t3.rearrange("p (k w) -> p k w", w=B2)
                nc.gpsimd.memset(p3[:, :, 0:CP], INF2)
                nc.gpsimd.memset(p3[:, :, CP + 256:B2], INF2)
                for pol in range(2):
                    for ih in range(2):
                        ps2 = psp.tile([128, 256], bf16, tag="ps2",
                                       name=f"ps2_{s}{pol}{ih}")
                        for jh in range(2):
                            k1 = pol * 2 + jh
                            src = acca[:, k1 * 256 + 128 * ih: k1 * 256 + 128 * ih + 128]
                            nc.tensor.transpose(ps2[:, 128 * jh:128 * jh + 128],
                                                src, ident[:])
                        k2 = pol * 2 + ih
                        nc.scalar.copy(t3[:, k2 * B2 + CP: k2 * B2 + CP + 256], ps2[:])

                # ---- pass B: full D2 (shifts along j)
                accb = wp.tile([128, 4 * 256], bf16, tag=f"accb_{s}", name=f"accb_{s}")
                minplus(nc, wp, t3, accb, wn_, wp_, f"b{s}")

                # ---- tail: d = sqrt(pos2) - sqrt(neg2); accum sigmoid(pred)*d
                sq = wp.tile([128, 4 * 256], f32, tag=f"sq_{s}", name=f"sq_{s}")
                nc.scalar.activation(sq[:], accb[:], AF.Sqrt)
                dt_ = wp.tile([128, 2 * 256], f32, tag=f"dt_{s}", name=f"dt_{s}")
                nc.vector.tensor_tensor(dt_[:], sq[:, 2 * 256:4 * 256],
                                        sq[:, 0:2 * 256], op=OP.subtract)
                pr = wp.tile([128, 2 * 256], f32, tag=f"pr_{s}", name=f"pr_{s}")
                # src pred[s, it*128+p, j] -> dst [p, (it, j)]
                pv2 = pred_h.ap()[s].rearrange("(t p) j -> p t j", p=128)
                pdma = nc.sync.dma_start(pr.rearrange("p (t j) -> p t j", t=2), pv2)
                # keep the pred DMA off the input-critical window
                add_dep_helper(pdma.ins, last_a.ins, sync=False,
                               reason="defer pred load behind pass A")
                sg = wp.tile([128, 2 * 256], f32, tag=f"sg_{s}", name=f"sg_{s}")
                nc.scalar.activation(sg[:], pr[:], AF.Sigmoid)
                dm = wp.tile([128, 2 * 256], f32, tag=f"dm_{s}", name=f"dm_{s}")
                nc.vector.tensor_tensor(dm[:], dt_[:], sg[:], op=OP.mult)
                scr = wp.tile([128, 2 * 256], f32, tag=f"scr_{s}", name=f"scr_{s}")
                nc.scalar.activation(scr[:], dm[:], AF.Copy,
                                     accum_out=outsb[:, s:s + 1])

            nc.sync.dma_start(out_h.ap(), outsb[:])
    nc.compile()
    return nc


# ---------------------------------------------------------------------------
# host side

def _row_dist(src):
    n, h, w = src.shape
    big = 10 ** 9
    col = np.arange(w)
    last = np.where(src, col, -big)
    np.maximum.accumulate(last, axis=2, out=last)
    nxt = np.where(src, col, big)
    nxt = np.minimum.accumulate(nxt[:, :, ::-1], axis=2)[:, :, ::-1]
    return np.minimum(np.minimum(col - last, nxt - col), big)


def _exact_d2(src):
    g = _row_dist(src).astype(np.int64)
    g2 = np.minimum(g * g, 10 ** 14)
    d2 = g2.copy()
    cur_max = d2.max()
    for d in range(1, src.shape[1]):
        v = d * d
        if v > cur_max:
            break
        np.minimum(d2[:, d:, :], g2[:, :-d, :] + v, out=d2[:, d:, :])
        np.minimum(d2[:, :-d, :], g2[:, d:, :] + v, out=d2[:, :-d, :])
        cur_max = d2.max()
    return d2


def _host_loss_f64(pred24, z24):
    d2n = _exact_d2(z24)
    d2p = _exact_d2(~z24)
    d = np.sqrt(d2p.astype(np.float64)) - np.sqrt(d2n.astype(np.float64))
    for m in range(z24.shape[0]):
        if not z24[m].any():
            d[m] = 0.0
    sig = 1.0 / (1.0 + np.exp(-pred24.astype(np.float64)))
    return np.float32((sig * d).mean())


def _plan(targ24):
    """Returns (per-slot neg windows, pos windows, mask order)."""
    z24 = targ24 != 0
    d2n = _exact_d2(z24).reshape(24, -1).max(1)
    d2p = _exact_d2(~z24).reshape(24, -1).max(1)
    wn = np.maximum(np.floor(np.sqrt(d2n)).astype(int), 1)
    wp_ = np.maximum(np.floor(np.sqrt(d2p)).astype(int), 1)
    wm = np.maximum(wn, wp_)
    order = np.argsort(-wm, kind="stable")
    swn = [0] * NMASK
    swp = [0] * NMASK
    for r, m in enumerate(order):
        s = r // NCORES
        swn[s] = max(swn[s], int(wn[m]))
        swp[s] = max(swp[s], int(wp_[m]))
    for s in range(NMASK - 2, -1, -1):
        swn[s] = max(swn[s], swn[s + 1])
        swp[s] = max(swp[s], swp[s + 1])
    # kernel assumes wpos >= wneg per slot (pos-only tail shifts)
    for s in range(NMASK):
        if swn[s] > swp[s]:
            swn[s], swp[s] = swp[s], swn[s]
    return swn, swp, order


def kernel(pred, target):
    pred24 = np.ascontiguousarray(np.asarray(pred, dtype=np.float32).reshape(24, H, W))
    targ24 = np.ascontiguousarray(np.asarray(target, dtype=np.int32).reshape(24, H, W))
    z24 = targ24 != 0

    if any((not z24[m].any()) or z24[m].all() for m in range(24)):
        return _host_loss_f64(pred24, z24)

    swn, swp, order = _plan(targ24)
    key = (tuple(swn), tuple(swp))
    if key not in _NC_CACHE:
        _NC_CACHE[key] = build_nc(swn, swp)
    nc = _NC_CACHE[key]

    in_maps = []
    for c in range(NCORES):
        midx = [order[s * NCORES + c] for s in range(NMASK)]
        in_maps.append({
            "pred": np.ascontiguousarray(pred24[midx]),
            "target": np.ascontiguousarray(targ24[midx]),
        })
    res = run_bass_kernel_spmd(nc, in_maps, core_ids=list(range(NCORES)))
    total = np.float64(0.0)
    for c in range(NCORES):
        total += np.asarray(res.results[c]["out"], dtype=np.float64).sum()
    return np.float32(total / (24.0 * H * W))
